# revision 23
# baseline (speedup 1.0000x reference)
"""Trainium2 Bass kernel for nn_Attention_68401649156342.

Reference computation (per batch element b of 8):
    q = MLP_q(x[b])                 # [2048,128] -> relu(x@Wq1+bq1)@Wq2+bq2 -> [2048,256]
    k = MLP_k(x[b])
    s = q @ k.T                     # [2048,2048]
    m = rowmax(s)
    out[b] = softmax(s / m, axis=-1)

Sharding: pure data-parallel over batch. Each of the 8 NeuronCores handles one
batch element end-to-end; no collectives.

Per-core dataflow (bf16 compute, f32 accumulate):
    - host pre-transposes x[b] -> xT [128(F),2048(S)], casts x/W to bf16, and
      packs biases + the exp bias constant into one [128,9] f32 block
    - a few dummy matmuls run during the input DMA to open the PE clock gate
    - MLP layer 1: hT[d,s] = relu(W1.T @ xT + b1)  (PE matmul, ScalarE epilogue)
    - MLP layer 2: qT/kT[d,s] = W2.T @ hT + b2     (PE matmul, VectorE epilogue)
      K side first: scores need all of kT but only a 128-col slice of qT
    - scores tile m (16 x [128,2048] f32 PSUM, 2 slots): qT_slice.T @ kT
    - VectorE: row-max from PSUM + reciprocal (mx is a bufs=1 tile so the
      scheduler keeps each reciprocal glued to its own reduce)
    - ScalarE: p = exp(scores * (1/max) - 1) PSUM->SBUF bf16, with fused
      row-sum accumulation
    - VectorE: o = p * (1/sum)  (bf16 4x mode)
    - DMA o (bf16) -> out[b]; host upcasts to f32
"""

import os
from contextlib import ExitStack

import ml_dtypes
import numpy as np

B, S, F, D = 8, 2048, 128, 256
NCORES = 8

FP32 = None  # set lazily in _build (mybir import deferred)

_CACHED = {}


def _build():
    import concourse.bass as bass
    import concourse.tile as tile
    from concourse import bacc, mybir

    f32 = mybir.dt.float32
    bf16 = mybir.dt.bfloat16
    AF = mybir.ActivationFunctionType
    OP = mybir.AluOpType

    nc = bacc.Bacc("TRN2", target_bir_lowering=False, debug=False,
                   num_devices=NCORES)

    xT_d = nc.dram_tensor("xT", [F, S], bf16, kind="ExternalInput")
    w1_d = nc.dram_tensor("W1", [2, F, D], bf16, kind="ExternalInput")
    # W2 pre-tiled on host: [side, ktile, 128, D]
    w2_d = nc.dram_tensor("W2", [2, 2, 128, D], bf16, kind="ExternalInput")
    # host-packed per-partition constants: cols 0-3 = b1[s][m], 4-7 = b2[s][m2],
    # col 8 = -1.0 (exp bias)
    bc_d = nc.dram_tensor("BC", [128, 9], f32, kind="ExternalInput")
    out_d = nc.dram_tensor("out", [S, S], bf16, kind="ExternalOutput")

    NT = S // 128   # 16 score row-tiles
    NCH = S // 512  # 4 free-dim chunks per 2048 span

    with tile.TileContext(nc) as tc, ExitStack() as ctx:
        persist = ctx.enter_context(tc.tile_pool(name="persist", bufs=1))
        hpool = ctx.enter_context(tc.tile_pool(name="hpool", bufs=1))
        psum = ctx.enter_context(
            tc.tile_pool(name="psum", bufs=2, space="PSUM"))
        ppool = ctx.enter_context(tc.tile_pool(name="ppool", bufs=4))
        stats = ctx.enter_context(tc.tile_pool(name="stats", bufs=4))

        # ---- constant / persistent loads ----
        bc = persist.tile([128, 9], f32, tag="bc")
        nc.sync.dma_start(bc[:], bc_d[:])

        def b1sb(s, m):
            return bc[:, 2 * s + m:2 * s + m + 1]

        def b2sb(s, m2):
            return bc[:, 4 + 2 * s + m2:4 + 2 * s + m2 + 1]

        neg1 = bc[:, 8:9]

        xT = persist.tile([F, S], bf16, tag="xT")
        nc.sync.dma_start(xT[:], xT_d[:])

        w1 = persist.tile([F, 2, D], bf16, tag="w1")
        nc.sync.dma_start(w1[:], w1_d.ap().rearrange("s p d -> p s d"))
        w2 = persist.tile([128, 2, 2, D], bf16, tag="w2")
        nc.sync.dma_start(w2[:], w2_d.ap().rearrange("s k p d -> p s k d"))

        # ---- PE warm-up: dummy matmuls run during the input-DMA wait so
        # the HAM clock-gate opens before the first real matmul ----
        warm = persist.tile([128, 512], bf16, tag="warm")
        nc.gpsimd.memset(warm[:], 0.0)
        wps = psum.tile([128, S], f32, tag="ps", name="wps")
        for i in range(8):
            nc.tensor.matmul(wps[:, 0:512], warm[:, 0:128], warm[:],
                             start=True, stop=True)

        # ---- MLPs: produce qT/kT [2][128, S] bf16 (partition = feature d) ----
        # K side (s=1) first: every scores tile needs the full kT, while qT is
        # consumed in 128-col slices.
        qk = [[None, None], [None, None]]  # [side][dtile]
        for s in (1, 0):  # k side, then q side
            h = [None, None]
            for m in range(2):
                ps = psum.tile([128, S], f32, tag="ps")
                for n in range(NCH):
                    nc.tensor.matmul(
                        ps[:, n * 512:(n + 1) * 512],
                        w1[:, s, m * 128:(m + 1) * 128],
                        xT[:, n * 512:(n + 1) * 512],
                        start=True, stop=True)
                h[m] = hpool.tile([128, S], bf16, tag=f"h{m}", name=f"h_{s}_{m}")
                # relu(ps + b1) -> bf16, on ScalarE, in halves: subtile deps
                # let layer 2 start on the first half while the second half
                # is still converting
                for hf in range(2):
                    sl = slice(hf * 1024, (hf + 1) * 1024)
                    nc.scalar.activation(h[m][:, sl], ps[:, sl], AF.Relu,
                                         bias=b1sb(s, m), scale=1.0)
            for m2 in range(2):
                ps2 = psum.tile([128, S], f32, tag="ps")
                for k in range(2):
                    for n in range(NCH):
                        nc.tensor.matmul(
                            ps2[:, n * 512:(n + 1) * 512],
                            w2[:, s, k, m2 * 128:(m2 + 1) * 128],
                            h[k][:, n * 512:(n + 1) * 512],
                            start=(k == 0), stop=(k == 1))
                qk[s][m2] = persist.tile([128, S], bf16, tag=f"qk{s}{m2}", name=f"qk_{s}_{m2}")
                # ps2 + b2 -> bf16, on VectorE (ScalarE owns the relu ops).
                # K side in halves for the same subtile-dep reason; Q side
                # stays one op -- it borders the scores phase, where an extra
                # DVE op in front of the first reduce costs more than it saves.
                if s == 1:
                    for hf in range(2):
                        sl = slice(hf * 1024, (hf + 1) * 1024)
                        nc.vector.tensor_scalar_add(qk[s][m2][:, sl],
                                                    ps2[:, sl], b2sb(s, m2))
                else:
                    nc.vector.tensor_scalar_add(qk[s][m2][:], ps2[:],
                                                b2sb(s, m2))

        q, k = qk[0], qk[1]

        # ---- scores + softmax, tile by tile ----
        for m in range(NT):
            ps = psum.tile([128, S], f32, tag="ps")
            for kk in range(2):
                for n in range(NCH):
                    nc.tensor.matmul(
                        ps[:, n * 512:(n + 1) * 512],
                        q[kk][:, m * 128:(m + 1) * 128],
                        k[kk][:, n * 512:(n + 1) * 512],
                        start=(kk == 0), stop=(kk == 1))

            # mx has bufs=1: the next tile's reduce can only claim the slot
            # after this tile's reciprocal consumed mx, which forces the
            # scheduler to place each reciprocal right after its own reduce
            # (otherwise the exp stalls ~2.3us behind an unrelated reduce).
            mx = stats.tile([128, 1], f32, tag="mx", bufs=1)
            r = stats.tile([128, 1], f32, tag="r")
            nc.vector.reduce_max(mx[:], ps[:], axis=mybir.AxisListType.X)
            nc.vector.reciprocal(r[:], mx[:])
            p = ppool.tile([128, S], bf16, tag="p")
            ssum = stats.tile([128, 1], f32, tag="ssum")
            # p = exp(ps * (1/mx) - 1), ssum = rowsum(p)   [one ACT pass, PSUM->SBUF]
            nc.scalar.activation(p[:], ps[:], AF.Exp,
                                 bias=neg1, scale=r[:], accum_out=ssum[:])
            rs = stats.tile([128, 1], f32, tag="rs")
            nc.vector.reciprocal(rs[:], ssum[:])
            o = ppool.tile([128, S], bf16, tag="o")
            # o = p * (1/sum)  [DVE 4x bf16; cheap enough to keep off ACT,
            # whose queue would stall the next exp]
            nc.vector.tensor_scalar_mul(o[:], p[:], rs[:])

            nc.sync.dma_start(out_d[m * 128:(m + 1) * 128, :], o[:])

    nc.compile()
    return nc


def _get_nc():
    if "nc" not in _CACHED:
        _CACHED["nc"] = _build()
    return _CACHED["nc"]


def _prep_inputs(x, Wq1, bq1, Wq2, bq2, Wk1, bk1, Wk2, bk2):
    bf = ml_dtypes.bfloat16
    xT = np.ascontiguousarray(x.transpose(0, 2, 1)).astype(bf)  # [B,F,S]
    W1 = np.ascontiguousarray(np.stack([Wq1, Wk1])).astype(bf)  # [2,F,D]
    W2 = np.ascontiguousarray(
        np.stack([Wq2.reshape(2, 128, D), Wk2.reshape(2, 128, D)])).astype(bf)
    # per-partition const block: cols 0-3 = b1[s][m], 4-7 = b2[s][m2], 8 = -1
    BC = np.empty((128, 9), np.float32)
    for s, (b1v, b2v) in enumerate([(bq1, bq2), (bk1, bk2)]):
        for mm in range(2):
            BC[:, 2 * s + mm] = np.asarray(b1v)[mm * 128:(mm + 1) * 128]
            BC[:, 4 + 2 * s + mm] = np.asarray(b2v)[mm * 128:(mm + 1) * 128]
    BC[:, 8] = -1.0
    BC = np.ascontiguousarray(BC)
    return [
        {"xT": np.ascontiguousarray(xT[b]), "W1": W1, "W2": W2, "BC": BC}
        for b in range(B)
    ]


def _ensure_trace_hook():
    """Provide antenv.axon_hooks (NTFF profiling hook) if the image lacks it.

    Only matters when BASS_TRACE=1; degrades silently otherwise.
    """
    import sys
    import types
    try:
        import antenv.axon_hooks  # noqa: F401
        return
    except ImportError:
        pass
    try:
        import antenv
        from trn_agent_boot.trn_boot import _ntff_profile_via_ctypes

        mod = types.ModuleType("antenv.axon_hooks")
        state = {"hook": _ntff_profile_via_ctypes("/opt/axon/libaxon_pjrt.so")}
        mod.set_axon_ntff_profile_hook = lambda h: state.update(hook=h)
        mod.get_axon_ntff_profile_hook = lambda: state["hook"]
        sys.modules["antenv.axon_hooks"] = mod
        antenv.axon_hooks = mod
    except Exception:
        pass


def kernel(x, Wq1, bq1, Wq2, bq2, Wk1, bk1, Wk2, bk2):
    from concourse.bass_utils import run_bass_kernel_spmd

    try:
        _ensure_trace_hook()
    except Exception:
        pass

    nc = _get_nc()
    in_maps = _prep_inputs(x, Wq1, bq1, Wq2, bq2, Wk1, bk1, Wk2, bk2)
    res = run_bass_kernel_spmd(nc, in_maps, core_ids=list(range(NCORES)))
    _CACHED["last_results"] = res
    if res.exec_time_ns is not None:
        print(f"HW exec time: {res.exec_time_ns} ns")
    out = np.stack([res.results[b]["out"] for b in range(B)])
    # kernel computes/stores in bf16; deliver f32 to match the reference dtype
    return out.astype(np.float32)


# revision 24
# speedup vs baseline: 1.1963x; 1.1963x over previous
"""Trainium2 Bass kernel for nn_Attention_68401649156342.

Reference computation (per batch element b of 8):
    q = MLP_q(x[b])                 # [2048,128] -> relu(x@Wq1+bq1)@Wq2+bq2 -> [2048,256]
    k = MLP_k(x[b])
    s = q @ k.T                     # [2048,2048]
    m = rowmax(s)
    out[b] = softmax(s / m, axis=-1)

Sharding: pure data-parallel over batch. Each of the 8 NeuronCores handles one
batch element end-to-end; no collectives.

Per-core dataflow (bf16 compute, f32 accumulate):
    - host pre-transposes x[b] -> xT [128(F),2048(S)], casts x/W to bf16, and
      packs biases + the exp bias constant into one [128,9] f32 block
    - a few dummy matmuls run during the input DMA to open the PE clock gate
    - MLP layer 1: hT[d,s] = relu(W1.T @ xT + b1)  (PE matmul, ScalarE epilogue)
    - MLP layer 2: qT/kT[d,s] = W2.T @ hT + b2     (PE matmul, VectorE epilogue)
      K side first: scores need all of kT but only a 128-col slice of qT
    - scores tile m (16 x [128,2048] f32 PSUM, 2 slots): qT_slice.T @ kT
    - VectorE: row-max from PSUM + reciprocal (mx is a bufs=1 tile so the
      scheduler keeps each reciprocal glued to its own reduce)
    - ScalarE: p = exp(scores * (1/max) - 1) PSUM->SBUF bf16, with fused
      row-sum accumulation
    - VectorE: o = p * (1/sum)  (bf16 4x mode)
    - DMA o (bf16) -> out[b]; host upcasts to f32
"""

import os
from contextlib import ExitStack

import ml_dtypes
import numpy as np

B, S, F, D = 8, 2048, 128, 256
NCORES = 8

FP32 = None  # set lazily in _build (mybir import deferred)

_CACHED = {}


def _build():
    import concourse.bass as bass
    import concourse.tile as tile
    from concourse import bacc, mybir

    f32 = mybir.dt.float32
    bf16 = mybir.dt.bfloat16
    AF = mybir.ActivationFunctionType
    OP = mybir.AluOpType

    nc = bacc.Bacc("TRN2", target_bir_lowering=False, debug=False,
                   num_devices=NCORES)

    xT_d = nc.dram_tensor("xT", [F, S], bf16, kind="ExternalInput")
    w1_d = nc.dram_tensor("W1", [2, F, D], bf16, kind="ExternalInput")
    # W2 pre-tiled on host: [side, ktile, 128, D]
    w2_d = nc.dram_tensor("W2", [2, 2, 128, D], bf16, kind="ExternalInput")
    # host-packed per-partition constants: cols 0-3 = b1[s][m], 4-7 = b2[s][m2],
    # col 8 = -1.0 (exp bias)
    bc_d = nc.dram_tensor("BC", [128, 9], f32, kind="ExternalInput")
    out_d = nc.dram_tensor("out", [S, S], bf16, kind="ExternalOutput")

    NT = S // 128   # 16 score row-tiles
    NCH = S // 512  # 4 free-dim chunks per 2048 span

    with tile.TileContext(nc) as tc, ExitStack() as ctx:
        persist = ctx.enter_context(tc.tile_pool(name="persist", bufs=1))
        hpool = ctx.enter_context(tc.tile_pool(name="hpool", bufs=1))
        psum = ctx.enter_context(
            tc.tile_pool(name="psum", bufs=2, space="PSUM"))
        ppool = ctx.enter_context(tc.tile_pool(name="ppool", bufs=4))
        stats = ctx.enter_context(tc.tile_pool(name="stats", bufs=4))

        # ---- constant / persistent loads ----
        bc = persist.tile([128, 9], f32, tag="bc")
        nc.sync.dma_start(bc[:], bc_d[:])

        def b1sb(s, m):
            return bc[:, 2 * s + m:2 * s + m + 1]

        def b2sb(s, m2):
            return bc[:, 4 + 2 * s + m2:4 + 2 * s + m2 + 1]

        neg1 = bc[:, 8:9]

        xT = persist.tile([F, S], bf16, tag="xT")
        nc.sync.dma_start(xT[:], xT_d[:])

        w1 = persist.tile([F, 2, D], bf16, tag="w1")
        nc.sync.dma_start(w1[:], w1_d.ap().rearrange("s p d -> p s d"))
        w2 = persist.tile([128, 2, 2, D], bf16, tag="w2")
        nc.sync.dma_start(w2[:], w2_d.ap().rearrange("s k p d -> p s k d"))

        # ---- PE warm-up: dummy matmuls run during the input-DMA wait so
        # the HAM clock-gate opens before the first real matmul ----
        warm = persist.tile([128, 512], bf16, tag="warm")
        nc.gpsimd.memset(warm[:], 0.0)
        wps = psum.tile([128, S], f32, tag="ps", name="wps")
        for i in range(8):
            nc.tensor.matmul(wps[:, 0:512], warm[:, 0:128], warm[:],
                             start=True, stop=True)

        # ---- MLPs: produce qT/kT [2][128, S] bf16 (partition = feature d) ----
        # K side (s=1) first: every scores tile needs the full kT, while qT is
        # consumed in 128-col slices.
        qk = [[None, None], [None, None]]  # [side][dtile]
        for s in (1, 0):  # k side, then q side
            h = [None, None]
            for m in range(2):
                ps = psum.tile([128, S], f32, tag="ps")
                for n in range(NCH):
                    nc.tensor.matmul(
                        ps[:, n * 512:(n + 1) * 512],
                        w1[:, s, m * 128:(m + 1) * 128],
                        xT[:, n * 512:(n + 1) * 512],
                        start=True, stop=True)
                h[m] = hpool.tile([128, S], bf16, tag=f"h{m}", name=f"h_{s}_{m}")
                # relu(ps + b1) -> bf16, on ScalarE
                nc.scalar.activation(h[m][:], ps[:], AF.Relu,
                                     bias=b1sb(s, m), scale=1.0)
            for m2 in range(2):
                ps2 = psum.tile([128, S], f32, tag="ps")
                for k in range(2):
                    for n in range(NCH):
                        nc.tensor.matmul(
                            ps2[:, n * 512:(n + 1) * 512],
                            w2[:, s, k, m2 * 128:(m2 + 1) * 128],
                            h[k][:, n * 512:(n + 1) * 512],
                            start=(k == 0), stop=(k == 1))
                qk[s][m2] = persist.tile([128, S], bf16, tag=f"qk{s}{m2}", name=f"qk_{s}_{m2}")
                # ps2 + b2 -> bf16, on VectorE (ScalarE owns the relu ops)
                nc.vector.tensor_scalar_add(qk[s][m2][:], ps2[:],
                                            b2sb(s, m2))

        q, k = qk[0], qk[1]

        # ---- scores + softmax, tile by tile ----
        for m in range(NT):
            ps = psum.tile([128, S], f32, tag="ps")
            for kk in range(2):
                for n in range(NCH):
                    nc.tensor.matmul(
                        ps[:, n * 512:(n + 1) * 512],
                        q[kk][:, m * 128:(m + 1) * 128],
                        k[kk][:, n * 512:(n + 1) * 512],
                        start=(kk == 0), stop=(kk == 1))

            # mx has bufs=1: the next tile's reduce can only claim the slot
            # after this tile's reciprocal consumed mx, which forces the
            # scheduler to place each reciprocal right after its own reduce
            # (otherwise the exp stalls ~2.3us behind an unrelated reduce).
            mx = stats.tile([128, 1], f32, tag="mx", bufs=1)
            r = stats.tile([128, 1], f32, tag="r")
            nc.vector.reduce_max(mx[:], ps[:], axis=mybir.AxisListType.X)
            nc.vector.reciprocal(r[:], mx[:])
            p = ppool.tile([128, S], bf16, tag="p")
            ssum = stats.tile([128, 1], f32, tag="ssum")
            # p = exp(ps * (1/mx) - 1), ssum = rowsum(p)   [one ACT pass, PSUM->SBUF]
            nc.scalar.activation(p[:], ps[:], AF.Exp,
                                 bias=neg1, scale=r[:], accum_out=ssum[:])
            rs = stats.tile([128, 1], f32, tag="rs")
            nc.vector.reciprocal(rs[:], ssum[:])
            o = ppool.tile([128, S], bf16, tag="o")
            # o = p * (1/sum)  [DVE 4x bf16; cheap enough to keep off ACT,
            # whose queue would stall the next exp]
            nc.vector.tensor_scalar_mul(o[:], p[:], rs[:])

            nc.sync.dma_start(out_d[m * 128:(m + 1) * 128, :], o[:])

    nc.compile()
    return nc


def _get_nc():
    if "nc" not in _CACHED:
        _CACHED["nc"] = _build()
    return _CACHED["nc"]


def _prep_inputs(x, Wq1, bq1, Wq2, bq2, Wk1, bk1, Wk2, bk2):
    bf = ml_dtypes.bfloat16
    xT = np.ascontiguousarray(x.transpose(0, 2, 1)).astype(bf)  # [B,F,S]
    W1 = np.ascontiguousarray(np.stack([Wq1, Wk1])).astype(bf)  # [2,F,D]
    W2 = np.ascontiguousarray(
        np.stack([Wq2.reshape(2, 128, D), Wk2.reshape(2, 128, D)])).astype(bf)
    # per-partition const block: cols 0-3 = b1[s][m], 4-7 = b2[s][m2], 8 = -1
    BC = np.empty((128, 9), np.float32)
    for s, (b1v, b2v) in enumerate([(bq1, bq2), (bk1, bk2)]):
        for mm in range(2):
            BC[:, 2 * s + mm] = np.asarray(b1v)[mm * 128:(mm + 1) * 128]
            BC[:, 4 + 2 * s + mm] = np.asarray(b2v)[mm * 128:(mm + 1) * 128]
    BC[:, 8] = -1.0
    BC = np.ascontiguousarray(BC)
    return [
        {"xT": np.ascontiguousarray(xT[b]), "W1": W1, "W2": W2, "BC": BC}
        for b in range(B)
    ]


def _ensure_trace_hook():
    """Provide antenv.axon_hooks (NTFF profiling hook) if the image lacks it.

    Only matters when BASS_TRACE=1; degrades silently otherwise.
    """
    import sys
    import types
    try:
        import antenv.axon_hooks  # noqa: F401
        return
    except ImportError:
        pass
    try:
        import antenv
        from trn_agent_boot.trn_boot import _ntff_profile_via_ctypes

        mod = types.ModuleType("antenv.axon_hooks")
        state = {"hook": _ntff_profile_via_ctypes("/opt/axon/libaxon_pjrt.so")}
        mod.set_axon_ntff_profile_hook = lambda h: state.update(hook=h)
        mod.get_axon_ntff_profile_hook = lambda: state["hook"]
        sys.modules["antenv.axon_hooks"] = mod
        antenv.axon_hooks = mod
    except Exception:
        pass


def kernel(x, Wq1, bq1, Wq2, bq2, Wk1, bk1, Wk2, bk2):
    from concourse.bass_utils import run_bass_kernel_spmd

    try:
        _ensure_trace_hook()
    except Exception:
        pass

    nc = _get_nc()
    in_maps = _prep_inputs(x, Wq1, bq1, Wq2, bq2, Wk1, bk1, Wk2, bk2)
    res = run_bass_kernel_spmd(nc, in_maps, core_ids=list(range(NCORES)))
    _CACHED["last_results"] = res
    if res.exec_time_ns is not None:
        print(f"HW exec time: {res.exec_time_ns} ns")
    out = np.stack([res.results[b]["out"] for b in range(B)])
    # kernel computes/stores in bf16; deliver f32 to match the reference dtype
    return out.astype(np.float32)


# revision 25
# speedup vs baseline: 1.1984x; 1.0018x over previous
"""Trainium2 Bass kernel for nn_Attention_68401649156342.

Reference computation (per batch element b of 8):
    q = MLP_q(x[b])                 # [2048,128] -> relu(x@Wq1+bq1)@Wq2+bq2 -> [2048,256]
    k = MLP_k(x[b])
    s = q @ k.T                     # [2048,2048]
    m = rowmax(s)
    out[b] = softmax(s / m, axis=-1)

Sharding: pure data-parallel over batch. Each of the 8 NeuronCores handles one
batch element end-to-end; no collectives.

Per-core dataflow (bf16 compute, f32 accumulate):
    - host pre-transposes x[b] -> xT [128(F),2048(S)], casts x/W to bf16, and
      packs biases + the exp bias constant into one [128,9] f32 block
    - a few dummy matmuls run during the input DMA to open the PE clock gate
    - MLP layer 1: hT[d,s] = relu(W1.T @ xT + b1)  (PE matmul, ScalarE epilogue)
    - MLP layer 2: qT/kT[d,s] = W2.T @ hT + b2     (PE matmul, VectorE epilogue)
      K side first: scores need all of kT but only a 128-col slice of qT
    - scores tile m (16 x [128,2048] f32 PSUM, 2 slots): qT_slice.T @ kT
    - VectorE: row-max from PSUM + reciprocal (mx is a bufs=1 tile so the
      scheduler keeps each reciprocal glued to its own reduce)
    - ScalarE: p = exp(scores * (1/max) - 1) PSUM->SBUF bf16, with fused
      row-sum accumulation
    - VectorE: o = p * (1/sum)  (bf16 4x mode)
    - DMA o (bf16) -> out[b]; host upcasts to f32
"""

import os
from contextlib import ExitStack

import ml_dtypes
import numpy as np

B, S, F, D = 8, 2048, 128, 256
NCORES = 8

FP32 = None  # set lazily in _build (mybir import deferred)

_CACHED = {}


def _build():
    import concourse.bass as bass
    import concourse.tile as tile
    from concourse import bacc, mybir

    f32 = mybir.dt.float32
    bf16 = mybir.dt.bfloat16
    AF = mybir.ActivationFunctionType
    OP = mybir.AluOpType

    nc = bacc.Bacc("TRN2", target_bir_lowering=False, debug=False,
                   num_devices=NCORES)

    xT_d = nc.dram_tensor("xT", [F, S], bf16, kind="ExternalInput")
    w1_d = nc.dram_tensor("W1", [2, F, D], bf16, kind="ExternalInput")
    # W2 pre-tiled on host: [side, ktile, 128, D]
    w2_d = nc.dram_tensor("W2", [2, 2, 128, D], bf16, kind="ExternalInput")
    # host-packed per-partition constants: cols 0-3 = b1[s][m], 4-7 = b2[s][m2],
    # col 8 = -1.0 (exp bias)
    bc_d = nc.dram_tensor("BC", [128, 9], f32, kind="ExternalInput")
    out_d = nc.dram_tensor("out", [S, S], bf16, kind="ExternalOutput")

    NT = S // 128   # 16 score row-tiles
    NCH = S // 512  # 4 free-dim chunks per 2048 span

    with tile.TileContext(nc) as tc, ExitStack() as ctx:
        persist = ctx.enter_context(tc.tile_pool(name="persist", bufs=1))
        hpool = ctx.enter_context(tc.tile_pool(name="hpool", bufs=1))
        psum = ctx.enter_context(
            tc.tile_pool(name="psum", bufs=2, space="PSUM"))
        ppool = ctx.enter_context(tc.tile_pool(name="ppool", bufs=6))
        stats = ctx.enter_context(tc.tile_pool(name="stats", bufs=6))

        # ---- constant / persistent loads ----
        bc = persist.tile([128, 9], f32, tag="bc")
        nc.sync.dma_start(bc[:], bc_d[:])

        def b1sb(s, m):
            return bc[:, 2 * s + m:2 * s + m + 1]

        def b2sb(s, m2):
            return bc[:, 4 + 2 * s + m2:4 + 2 * s + m2 + 1]

        neg1 = bc[:, 8:9]

        xT = persist.tile([F, S], bf16, tag="xT")
        nc.sync.dma_start(xT[:], xT_d[:])

        w1 = persist.tile([F, 2, D], bf16, tag="w1")
        nc.sync.dma_start(w1[:], w1_d.ap().rearrange("s p d -> p s d"))
        w2 = persist.tile([128, 2, 2, D], bf16, tag="w2")
        nc.sync.dma_start(w2[:], w2_d.ap().rearrange("s k p d -> p s k d"))

        # ---- PE warm-up: dummy matmuls run during the input-DMA wait so
        # the HAM clock-gate opens before the first real matmul ----
        warm = persist.tile([128, 512], bf16, tag="warm")
        nc.gpsimd.memset(warm[:], 0.0)
        wps = psum.tile([128, S], f32, tag="ps", name="wps")
        for i in range(8):
            nc.tensor.matmul(wps[:, 0:512], warm[:, 0:128], warm[:],
                             start=True, stop=True)

        # ---- MLPs: produce qT/kT [2][128, S] bf16 (partition = feature d) ----
        # K side (s=1) first: every scores tile needs the full kT, while qT is
        # consumed in 128-col slices.
        qk = [[None, None], [None, None]]  # [side][dtile]
        for s in (1, 0):  # k side, then q side
            h = [None, None]
            for m in range(2):
                ps = psum.tile([128, S], f32, tag="ps")
                for n in range(NCH):
                    nc.tensor.matmul(
                        ps[:, n * 512:(n + 1) * 512],
                        w1[:, s, m * 128:(m + 1) * 128],
                        xT[:, n * 512:(n + 1) * 512],
                        start=True, stop=True)
                h[m] = hpool.tile([128, S], bf16, tag=f"h{m}", name=f"h_{s}_{m}")
                # relu(ps + b1) -> bf16, on ScalarE
                nc.scalar.activation(h[m][:], ps[:], AF.Relu,
                                     bias=b1sb(s, m), scale=1.0)
            for m2 in range(2):
                ps2 = psum.tile([128, S], f32, tag="ps")
                for k in range(2):
                    for n in range(NCH):
                        nc.tensor.matmul(
                            ps2[:, n * 512:(n + 1) * 512],
                            w2[:, s, k, m2 * 128:(m2 + 1) * 128],
                            h[k][:, n * 512:(n + 1) * 512],
                            start=(k == 0), stop=(k == 1))
                qk[s][m2] = persist.tile([128, S], bf16, tag=f"qk{s}{m2}", name=f"qk_{s}_{m2}")
                # ps2 + b2 -> bf16, on VectorE (ScalarE owns the relu ops)
                nc.vector.tensor_scalar_add(qk[s][m2][:], ps2[:],
                                            b2sb(s, m2))

        q, k = qk[0], qk[1]

        # ---- scores + softmax, tile by tile ----
        for m in range(NT):
            ps = psum.tile([128, S], f32, tag="ps")
            for kk in range(2):
                for n in range(NCH):
                    nc.tensor.matmul(
                        ps[:, n * 512:(n + 1) * 512],
                        q[kk][:, m * 128:(m + 1) * 128],
                        k[kk][:, n * 512:(n + 1) * 512],
                        start=(kk == 0), stop=(kk == 1))

            # mx has bufs=1: the next tile's reduce can only claim the slot
            # after this tile's reciprocal consumed mx, which forces the
            # scheduler to place each reciprocal right after its own reduce
            # (otherwise the exp stalls ~2.3us behind an unrelated reduce).
            mx = stats.tile([128, 1], f32, tag="mx", bufs=1)
            r = stats.tile([128, 1], f32, tag="r")
            nc.vector.reduce_max(mx[:], ps[:], axis=mybir.AxisListType.X)
            nc.vector.reciprocal(r[:], mx[:])
            p = ppool.tile([128, S], bf16, tag="p")
            ssum = stats.tile([128, 1], f32, tag="ssum")
            # p = exp(ps * (1/mx) - 1), ssum = rowsum(p)   [one ACT pass, PSUM->SBUF]
            nc.scalar.activation(p[:], ps[:], AF.Exp,
                                 bias=neg1, scale=r[:], accum_out=ssum[:])
            rs = stats.tile([128, 1], f32, tag="rs")
            nc.vector.reciprocal(rs[:], ssum[:])
            o = ppool.tile([128, S], bf16, tag="o")
            # o = p * (1/sum)  [DVE 4x bf16; cheap enough to keep off ACT,
            # whose queue would stall the next exp]
            nc.vector.tensor_scalar_mul(o[:], p[:], rs[:])

            nc.sync.dma_start(out_d[m * 128:(m + 1) * 128, :], o[:])

    nc.compile()
    return nc


def _get_nc():
    if "nc" not in _CACHED:
        _CACHED["nc"] = _build()
    return _CACHED["nc"]


def _prep_inputs(x, Wq1, bq1, Wq2, bq2, Wk1, bk1, Wk2, bk2):
    bf = ml_dtypes.bfloat16
    xT = np.ascontiguousarray(x.transpose(0, 2, 1)).astype(bf)  # [B,F,S]
    W1 = np.ascontiguousarray(np.stack([Wq1, Wk1])).astype(bf)  # [2,F,D]
    W2 = np.ascontiguousarray(
        np.stack([Wq2.reshape(2, 128, D), Wk2.reshape(2, 128, D)])).astype(bf)
    # per-partition const block: cols 0-3 = b1[s][m], 4-7 = b2[s][m2], 8 = -1
    BC = np.empty((128, 9), np.float32)
    for s, (b1v, b2v) in enumerate([(bq1, bq2), (bk1, bk2)]):
        for mm in range(2):
            BC[:, 2 * s + mm] = np.asarray(b1v)[mm * 128:(mm + 1) * 128]
            BC[:, 4 + 2 * s + mm] = np.asarray(b2v)[mm * 128:(mm + 1) * 128]
    BC[:, 8] = -1.0
    BC = np.ascontiguousarray(BC)
    return [
        {"xT": np.ascontiguousarray(xT[b]), "W1": W1, "W2": W2, "BC": BC}
        for b in range(B)
    ]


def _ensure_trace_hook():
    """Provide antenv.axon_hooks (NTFF profiling hook) if the image lacks it.

    Only matters when BASS_TRACE=1; degrades silently otherwise.
    """
    import sys
    import types
    try:
        import antenv.axon_hooks  # noqa: F401
        return
    except ImportError:
        pass
    try:
        import antenv
        from trn_agent_boot.trn_boot import _ntff_profile_via_ctypes

        mod = types.ModuleType("antenv.axon_hooks")
        state = {"hook": _ntff_profile_via_ctypes("/opt/axon/libaxon_pjrt.so")}
        mod.set_axon_ntff_profile_hook = lambda h: state.update(hook=h)
        mod.get_axon_ntff_profile_hook = lambda: state["hook"]
        sys.modules["antenv.axon_hooks"] = mod
        antenv.axon_hooks = mod
    except Exception:
        pass


def kernel(x, Wq1, bq1, Wq2, bq2, Wk1, bk1, Wk2, bk2):
    from concourse.bass_utils import run_bass_kernel_spmd

    try:
        _ensure_trace_hook()
    except Exception:
        pass

    nc = _get_nc()
    in_maps = _prep_inputs(x, Wq1, bq1, Wq2, bq2, Wk1, bk1, Wk2, bk2)
    res = run_bass_kernel_spmd(nc, in_maps, core_ids=list(range(NCORES)))
    _CACHED["last_results"] = res
    if res.exec_time_ns is not None:
        print(f"HW exec time: {res.exec_time_ns} ns")
    out = np.stack([res.results[b]["out"] for b in range(B)])
    # kernel computes/stores in bf16; deliver f32 to match the reference dtype
    return out.astype(np.float32)
